# revision 11
# baseline (speedup 1.0000x reference)
"""Trainium2 Bass kernel for nn_Attention_73375221285454.

Multi-head self-attention (B=4, N=2048, D=768, H=12, DH=64) with key-padding
mask, distributed over 8 NeuronCores.

Sharding (head-split, tensor-parallel): core c handles batch b = c//2 and
head half hg = c%2 (6 of 12 heads: columns hg*384..hg*384+384 of Wq/Wk/Wv and
rows hg*384.. of Wo). Each core computes its 6 heads' Q/K/V projections,
attention, and a PARTIAL output projection for the whole batch; the host sums
the two partial outputs of each pair. No K/V duplication, no collectives.

Token sort: attention is permutation-invariant over tokens, so the host sorts
each batch's tokens with unmasked first (queries and keys are the same token
set). Only na = ceil(max_unmasked/128)*128 columns are processed on device
(~1152 of 2048 for a 50% mask) — this halves matmul, exp, and copy volume.
Masked-query rows never touch the device: the reference gives them a uniform
softmax over ALL keys, i.e. out = (mean_j x[b,j] @ Wv) @ Wo, which the host
computes directly in numpy. Pad columns (na_real..na) hold leftover masked
tokens; they are masked as keys via the additive bias table and their query
rows are discarded by the host.

Device algorithm per core (all matmuls bf16, fp32 PSUM):
  qT = (Wq_h.T @ xs)  [384, na] bf16     kT likewise
  vaug[key, h, 0:64] = V, [.., 64] = 1.0 (fp8e4; ones row makes P@V also
                                          accumulate the softmax denominator)
  per head h, key tile jt:
    S^T[128 keys, na] = kT_h,jt.T @ qT_h          (PSUM fp32)
    P^T = exp(0.125*S^T + bias[key])  -> fp8e4    (ACT writes matmul-ready
         bias = -2 (active) / -30000 (pad key);    fp8 directly - nothing on
         the -2 shift cancels in normalization     the DVE critical path)
         and keeps exp < 240 = fp8e4 max)
  per head h, query chunk qc (after all jt):
    psO[66, qc] = sum_jt vaug_jt.T @ P^T_jt       (fp8 DoubleRow matmuls:
         row 64 of psO = denominator s             2 key tiles per pass,
    attnT_h[:, qc] = psO[0:64] * (1/s)             0.5 cycles/row)
  out_partial = attnT.T @ Wo_h  [na, 768] fp32 -> DMA straight from PSUM.

exp needs no max-subtraction: logits ~ N(0,1), biased by -2 so exp() stays
well under the fp8e4 max of 240; masked keys get exp(-30000) == 0 exactly;
the fp8 quantization of P appears in both numerator and denominator so it
largely cancels in the softmax.
"""

import sys

sys.path.insert(0, "/opt/trn_rl_repo")

import numpy as np
import ml_dtypes

import concourse.bass as bass  # noqa: F401
import concourse.mybir as mybir
import concourse.tile as tile
from concourse import bacc
from concourse.bass_utils import run_bass_kernel_spmd

P = 128
B, N, D = 4, 2048, 768
H, DH = 12, 64
HPC = H // 2            # heads per core
HD = HPC * DH           # 384 projected dims per core
DC = D // P             # 6 contraction chunks
HDT = HD // P           # 3 head-dim chunks of 128
SCALE = DH ** -0.5      # 0.125
EXP_SHIFT = 0.0
DHW = 72                # vaug per-head width: 6*72 B jt-stride is 16B-aligned
                        # (DoubleRow LdWeights requires 16B-aligned steps)        # keeps exp() < fp8e4 max (240) at ~7.5 sigma logits
MASK_NEG = -30000.0
BF16 = ml_dtypes.bfloat16

f32 = mybir.dt.float32
bf16 = mybir.dt.bfloat16
fp8 = mybir.dt.float8e4

_BUILD_CACHE = {}


def _chunks(total, step):
    out = []
    off = 0
    while off < total:
        sz = min(step, total - off)
        out.append((off, sz))
        off += sz
    return out



def build(njt: int) -> "bacc.Bacc":
    """Build the SPMD program for njt active key tiles (na = njt*128)."""
    if njt in _BUILD_CACHE:
        return _BUILD_CACHE[njt]

    na = njt * P
    njt_v = njt + (njt % 2)
    psw = ((na * 4 + 2047) // 2048) * 512   # na rounded up to PSUM banks
    pss_bufs = 2 if 2 * psw * 4 + 2 * 2048 <= 16384 else 1

    nc = bacc.Bacc()
    xsT_d = nc.declare_dram_parameter("xsT", [D, na], bf16, isOutput=False)
    wq_d = nc.declare_dram_parameter("wq", [D, HD], bf16, isOutput=False)
    wk_d = nc.declare_dram_parameter("wk", [D, HD], bf16, isOutput=False)
    wv_d = nc.declare_dram_parameter("wv", [D, HD], bf16, isOutput=False)
    wo_d = nc.declare_dram_parameter("woT", [HD, D], bf16, isOutput=False)
    # cmneg[p, t] = 0.0 if key (t*128+p) active else MASK_NEG
    cm_d = nc.declare_dram_parameter("cmneg", [P, njt], f32, isOutput=False)
    out_d = nc.declare_dram_parameter("out", [na, D], f32, isOutput=True)

    xs_r = xsT_d.rearrange("(c p) n -> p c n", p=P)
    wq_r = wq_d.rearrange("(c p) e -> p c e", p=P)
    wk_r = wk_d.rearrange("(c p) e -> p c e", p=P)
    wv_r = wv_d.rearrange("(c p) e -> p c e", p=P)
    wo_r = wo_d.rearrange("(c p) e -> p c e", p=P)

    col_ch = _chunks(na, 512)        # token column chunks (moving <= 512)
    n_qc = len(col_ch)

    with tile.TileContext(nc) as tc:
        with tc.tile_pool(name="persist", bufs=1) as persist:
            cmneg = persist.tile([P, njt], f32)
            xs = persist.tile([P, DC, na], bf16)
            wq_sb = persist.tile([P, DC, HD], bf16)
            wk_sb = persist.tile([P, DC, HD], bf16)
            wv_sb = persist.tile([P, DC, HD], bf16)
            wo_sb = persist.tile([P, HDT, D], bf16)
            for dc in range(DC):
                nc.sync.dma_start(out=xs[:, dc, :], in_=xs_r[:, dc, :])
            nc.sync.dma_start(out=wq_sb, in_=wq_r)
            nc.sync.dma_start(out=wk_sb, in_=wk_r)
            nc.sync.dma_start(out=cmneg, in_=cm_d.ap())
            nc.sync.dma_start(out=wv_sb, in_=wv_r)
            nc.sync.dma_start(out=wo_sb, in_=wo_r)

            qT = persist.tile([P, HDT, na], bf16)
            kT = persist.tile([P, HDT, na], bf16)
            attnT = persist.tile([P, HDT, na], bf16)
            vaug = persist.tile([P, njt_v, HPC, DHW], bf16)
            pts0 = persist.tile([P, njt_v, na], bf16)
            pts1 = persist.tile([P, njt_v, na], bf16)

            # vaug ones column 64 (softmax-sum row), zero pad cols
            nc.vector.memset(vaug[:, :, :, DH:DHW], 0.0)
            nc.vector.memset(vaug[:, 0:njt, :, DH : DH + 1], 1.0)

            attn_scope = nc.named_scope("main"); attn_scope.__enter__()
            psxp_cm = tc.tile_pool(name="psx", bufs=2, space="PSUM")
            psxp = psxp_cm.__enter__()
            nrm_cm = tc.tile_pool(name="nrm", bufs=3)
            nrm = nrm_cm.__enter__()
            psSp_cm = tc.tile_pool(name="psS", bufs=pss_bufs, space="PSUM")
            psSp = psSp_cm.__enter__()

            def proj_full(w_sb, dst, t):
                """Full-width projection of head-dim chunk t via the big pool."""
                ps = psSp.tile([P, psw], f32, tag="psS", name=f"proj{t}")
                for off, sz in col_ch:
                    for dc in range(DC):
                        nc.tensor.matmul(
                            ps[:, off : off + sz],
                            w_sb[:, dc, t * P : (t + 1) * P],
                            xs[:, dc, off : off + sz],
                            start=(dc == 0),
                            stop=(dc == DC - 1),
                        )
                nc.vector.tensor_copy(dst[:, t, :], ps[:, 0:na])

            # -- PE filler work, emitted piecewise between score tiles --
            filler = []

            def f_vproj(jt):
                def emit():
                    psv = psxp.tile([P, 512], f32, tag="psx", name=f"psv{jt}")
                    for dc in range(DC):
                        nc.tensor.matmul(
                            psv[:, 0:HD],
                            xs[:, dc, jt * P : (jt + 1) * P],
                            wv_sb[:, dc, :],
                            start=(dc == 0),
                            stop=(dc == DC - 1),
                        )
                    nc.vector.tensor_copy(
                        vaug[:, jt, :, 0:DH],
                        psv[:, 0:HD].rearrange("p (h d) -> p h d", h=HPC),
                    )
                return emit

            def f_projchunk(w_sb, dst, t, off, sz):
                def emit():
                    ps = psxp.tile([P, 512], f32, tag="psx",
                                   name=f"pc{t}_{off}")
                    for dc in range(DC):
                        nc.tensor.matmul(
                            ps[:, 0:sz],
                            w_sb[:, dc, t * P : (t + 1) * P],
                            xs[:, dc, off : off + sz],
                            start=(dc == 0),
                            stop=(dc == DC - 1),
                        )
                    nc.vector.tensor_copy(dst[:, t, off : off + sz], ps[:, 0:sz])
                return emit

            def f_vatp(h, i, jt):
                """One P@V accumulation step; allocates psO at jt==0 and
                drains (normalize into attnT) after jt==njt-1."""
                hdt, pb = h // 2, DH * (h % 2)
                pts = pts0 if h % 2 == 0 else pts1
                off, sz = col_ch[i]

                def emit():
                    if jt == 0:
                        psO_state[(h, i)] = psxp.tile(
                            [P, 512], f32, tag="psx", name=f"psO{h}_{i}"
                        )
                    psO = psO_state[(h, i)]
                    nc.tensor.matmul(
                        psO[0:DHW, 0:sz],
                        vaug[:, jt, h, :],
                        pts[:, jt, off : off + sz],
                        start=(jt == 0),
                        stop=(jt == njt - 1),
                    )
                    if jt == njt - 1:
                        # exact DVE reciprocal: slow (2.5us) but overlapped
                        # under the PE-bound stream; Ln/Exp on ACT would
                        # thrash the activation table (1.3us reload each)
                        r_row = nrm.tile([1, 512], f32, tag="r_row")
                        nc.vector.reciprocal(r_row[:, 0:sz], psO[DH : DH + 1, 0:sz])
                        rb = nrm.tile([DH, 512], f32, tag="rb")
                        nc.gpsimd.partition_broadcast(
                            rb[:, 0:sz], r_row[:, 0:sz], channels=DH
                        )
                        nc.vector.tensor_mul(
                            attnT[pb : pb + DH, hdt, off : off + sz],
                            psO[0:DH, 0:sz],
                            rb[:, 0:sz],
                        )
                        del psO_state[(h, i)]
                return emit

            psO_state = {}

            # head-dim chunk 0 of Q and K up front; the rest is filler
            proj_full(wq_sb, qT, 0)
            proj_full(wk_sb, kT, 0)
            for jt in range(njt):
                filler.append(f_vproj(jt))
            for t in (1, 2):
                for w_sb, dst in ((wq_sb, qT), (wk_sb, kT)):
                    for off, sz in col_ch:
                        filler.append(f_projchunk(w_sb, dst, t, off, sz))

            # ---------------- attention ----------------
            for h in range(HPC):
                hdt, pb = h // 2, DH * (h % 2)
                pts = pts0 if h % 2 == 0 else pts1
                if h > 0:   # P@V of the previous head becomes filler
                    for i in range(n_qc):
                        for jt in range(njt):
                            filler.append(f_vatp(h - 1, i, jt))
                for jt in range(njt):
                    psS = psSp.tile([P, psw], f32, tag="psS")
                    for off, sz in col_ch:
                        nc.tensor.matmul(
                            psS[:, off : off + sz],
                            kT[pb : pb + DH, hdt, jt * P : (jt + 1) * P],
                            qT[pb : pb + DH, hdt, off : off + sz],
                            start=True,
                            stop=True,
                        )
                    # drip filler to keep PE busy during the ACT-bound loop
                    budget = 3 if h > 0 else 2
                    for _ in range(min(budget, len(filler))):
                        filler.pop(0)()
                    nc.scalar.activation(
                        pts[:, jt, 0:na],
                        psS[:, 0:na],
                        mybir.ActivationFunctionType.Exp,
                        bias=cmneg[:, jt : jt + 1],
                        scale=SCALE,
                    )
            for f in filler:   # leftover filler (rare)
                f()

            psSp_cm.__exit__(None, None, None)

            # ------------- tail: last head's P@V + output projection -------
            with tc.tile_pool(name="psF", bufs=2, space="PSUM") as psFp, \
                 tc.tile_pool(name="fin", bufs=2) as fin:

                def oproj(it):
                    psF = psFp.tile([P, 1024], f32, tag="psF")
                    for off, sz in ((0, 512), (512, 256)):
                        for c in range(HDT):
                            nc.tensor.matmul(
                                psF[:, off : off + sz],
                                attnT[:, c, it * P : (it + 1) * P],
                                wo_sb[:, c, off : off + sz],
                                start=(c == 0),
                                stop=(c == HDT - 1),
                            )
                    out_sb = fin.tile([P, D], f32, tag="out_sb")
                    nc.scalar.copy(out_sb, psF[:, 0:D])
                    nc.sync.dma_start(
                        out=out_d.ap()[it * P : (it + 1) * P, :], in_=out_sb
                    )

                h = HPC - 1
                done_tiles = 0
                for i in range(n_qc):
                    for jt in range(njt):
                        f_vatp(h, i, jt)()
                    if i > 0:
                        # out-proj tiles fully insidenow-drained chunks
                        lim = col_ch[i - 1][0] + col_ch[i - 1][1]
                        while (done_tiles + 1) * P <= lim:
                            oproj(done_tiles)
                            done_tiles += 1
                while done_tiles < njt:
                    oproj(done_tiles)
                    done_tiles += 1

            nrm_cm.__exit__(None, None, None)
            psxp_cm.__exit__(None, None, None)
            attn_scope.__exit__(None, None, None)

    nc.compile()
    _BUILD_CACHE[njt] = nc
    return nc


def _marshal(x, x_mask, Wq, Wk, Wv, Wo):
    """Build per-core input maps. Returns (in_maps, njt, orders, counts)."""
    x = np.asarray(x, dtype=np.float32)
    x_mask = np.asarray(x_mask).astype(bool)
    Wq = np.asarray(Wq, dtype=np.float32)
    Wk = np.asarray(Wk, dtype=np.float32)
    Wv = np.asarray(Wv, dtype=np.float32)
    Wo = np.asarray(Wo, dtype=np.float32)

    orders = [np.argsort(~x_mask[b], kind="stable") for b in range(B)]
    counts = [int(x_mask[b].sum()) for b in range(B)]
    njt = max(1, -(-max(counts) // P))
    na = njt * P

    xsTs, cms = [], []
    for b in range(B):
        xs_sorted = x[b][orders[b][:na]]                 # [na, 768]
        xsTs.append(np.ascontiguousarray(xs_sorted.T.astype(BF16)))
        key_act = np.arange(na) < counts[b]
        cm = np.where(key_act, EXP_SHIFT, MASK_NEG).astype(np.float32)
        cms.append(np.ascontiguousarray(cm.reshape(njt, P).T))

    whs = []
    for hg in range(2):
        cols = slice(hg * HD, (hg + 1) * HD)
        whs.append({
            "wq": np.ascontiguousarray(Wq[:, cols].astype(BF16)),
            "wk": np.ascontiguousarray(Wk[:, cols].astype(BF16)),
            "wv": np.ascontiguousarray(Wv[:, cols].astype(BF16)),
            "woT": np.ascontiguousarray(Wo[cols, :].astype(BF16)),
        })

    in_maps = []
    for c in range(8):
        b, hg = c // 2, c % 2
        in_maps.append({
            "xsT": xsTs[b], "cmneg": cms[b], **whs[hg],
        })
    return in_maps, njt, orders, counts


def run(x, x_mask, Wq, Wk, Wv, Wo, trace=False, tmpdir=None):
    """Run on 8 cores; returns (full_output, BassKernelResults)."""
    x = np.asarray(x, dtype=np.float32)
    Wv_f = np.asarray(Wv, dtype=np.float32)
    Wo_f = np.asarray(Wo, dtype=np.float32)
    in_maps, njt, orders, counts = _marshal(x, x_mask, Wq, Wk, Wv, Wo)
    nc = build(njt)
    res = run_bass_kernel_spmd(
        nc, in_maps, core_ids=list(range(8)), trace=trace, tmpdir=tmpdir
    )
    out = np.empty((B, N, D), dtype=np.float32)
    for b in range(B):
        s = (res.results[2 * b]["out"].astype(np.float32)
             + res.results[2 * b + 1]["out"].astype(np.float32))
        nr = counts[b]
        out[b, orders[b][:nr]] = s[:nr]
        if nr < N:
            # masked queries: uniform softmax over ALL keys
            mu = x[b].astype(np.float64).mean(axis=0)
            urow = (mu @ Wv_f.astype(np.float64)) @ Wo_f.astype(np.float64)
            out[b, orders[b][nr:]] = urow.astype(np.float32)
    return out, res


def kernel(**inputs) -> np.ndarray:
    out, _ = run(
        inputs["x"], inputs["x_mask"],
        inputs["Wq"], inputs["Wk"], inputs["Wv"], inputs["Wo"],
        trace=False,
    )
    return out


# revision 12
# speedup vs baseline: 1.5542x; 1.5542x over previous
"""Trainium2 Bass kernel for nn_Attention_73375221285454.

Multi-head self-attention (B=4, N=2048, D=768, H=12, DH=64) with key-padding
mask, distributed over 8 NeuronCores.

Sharding (head-split, tensor-parallel): core c handles batch b = c//2 and
head half hg = c%2 (6 of 12 heads: columns hg*384..hg*384+384 of Wq/Wk/Wv and
rows hg*384.. of Wo). Each core computes its 6 heads' Q/K/V projections,
attention, and a PARTIAL output projection for the whole batch; the host sums
the two partial outputs of each pair. No K/V duplication, no collectives.

Token sort: attention is permutation-invariant over tokens, so the host sorts
each batch's tokens with unmasked first (queries and keys are the same token
set). Only na = ceil(max_unmasked/128)*128 columns are processed on device
(~1152 of 2048 for a 50% mask) — this halves matmul, exp, and copy volume.
Masked-query rows never touch the device: the reference gives them a uniform
softmax over ALL keys, i.e. out = (mean_j x[b,j] @ Wv) @ Wo, which the host
computes directly in numpy. Pad columns (na_real..na) hold leftover masked
tokens; they are masked as keys via the additive bias table and their query
rows are discarded by the host.

Device algorithm per core (all matmuls bf16, fp32 PSUM):
  qT = (Wq_h.T @ xs)  [384, na] bf16     kT likewise
  vaug[key, h, 0:64] = V, [.., 64] = 1.0 (fp8e4; ones row makes P@V also
                                          accumulate the softmax denominator)
  per head h, key tile jt:
    S^T[128 keys, na] = kT_h,jt.T @ qT_h          (PSUM fp32)
    P^T = exp(0.125*S^T + bias[key])  -> fp8e4    (ACT writes matmul-ready
         bias = -2 (active) / -30000 (pad key);    fp8 directly - nothing on
         the -2 shift cancels in normalization     the DVE critical path)
         and keeps exp < 240 = fp8e4 max)
  per head h, query chunk qc (after all jt):
    psO[66, qc] = sum_jt vaug_jt.T @ P^T_jt       (fp8 DoubleRow matmuls:
         row 64 of psO = denominator s             2 key tiles per pass,
    attnT_h[:, qc] = psO[0:64] * (1/s)             0.5 cycles/row)
  out_partial = attnT.T @ Wo_h  [na, 768] fp32 -> DMA straight from PSUM.

exp needs no max-subtraction: logits ~ N(0,1), biased by -2 so exp() stays
well under the fp8e4 max of 240; masked keys get exp(-30000) == 0 exactly;
the fp8 quantization of P appears in both numerator and denominator so it
largely cancels in the softmax.
"""

import sys

sys.path.insert(0, "/opt/trn_rl_repo")

import numpy as np
import ml_dtypes

import concourse.bass as bass  # noqa: F401
import concourse.mybir as mybir
import concourse.tile as tile
from concourse import bacc
from concourse.bass_utils import run_bass_kernel_spmd

P = 128
B, N, D = 4, 2048, 768
H, DH = 12, 64
HPC = H // 2            # heads per core
HD = HPC * DH           # 384 projected dims per core
DC = D // P             # 6 contraction chunks
HDT = HD // P           # 3 head-dim chunks of 128
SCALE = DH ** -0.5      # 0.125
EXP_SHIFT = 0.0
DHW = 72                # vaug per-head width: 6*72 B jt-stride is 16B-aligned
                        # (DoubleRow LdWeights requires 16B-aligned steps)        # keeps exp() < fp8e4 max (240) at ~7.5 sigma logits
MASK_NEG = -30000.0
BF16 = ml_dtypes.bfloat16

f32 = mybir.dt.float32
bf16 = mybir.dt.bfloat16
fp8 = mybir.dt.float8e4

_BUILD_CACHE = {}


def _chunks(total, step):
    out = []
    off = 0
    while off < total:
        sz = min(step, total - off)
        out.append((off, sz))
        off += sz
    return out



def build(njt: int) -> "bacc.Bacc":
    """Build the SPMD program for njt active key tiles (na = njt*128)."""
    if njt in _BUILD_CACHE:
        return _BUILD_CACHE[njt]

    na = njt * P
    njt_v = njt + (njt % 2)
    psw = ((na * 4 + 2047) // 2048) * 512   # na rounded up to PSUM banks
    pss_bufs = 2 if 2 * psw * 4 + 2 * 2048 <= 16384 else 1

    nc = bacc.Bacc()
    xsT_d = nc.declare_dram_parameter("xsT", [D, na], bf16, isOutput=False)
    wq_d = nc.declare_dram_parameter("wq", [D, HD], bf16, isOutput=False)
    wk_d = nc.declare_dram_parameter("wk", [D, HD], bf16, isOutput=False)
    wv_d = nc.declare_dram_parameter("wv", [D, HD], bf16, isOutput=False)
    wo_d = nc.declare_dram_parameter("woT", [HD, D], bf16, isOutput=False)
    # cmneg[p, t] = 0.0 if key (t*128+p) active else MASK_NEG
    cm_d = nc.declare_dram_parameter("cmneg", [P, njt], f32, isOutput=False)
    out_d = nc.declare_dram_parameter("out", [na, D], f32, isOutput=True)

    xs_r = xsT_d.rearrange("(c p) n -> p c n", p=P)
    wq_r = wq_d.rearrange("(c p) e -> p c e", p=P)
    wk_r = wk_d.rearrange("(c p) e -> p c e", p=P)
    wv_r = wv_d.rearrange("(c p) e -> p c e", p=P)
    wo_r = wo_d.rearrange("(c p) e -> p c e", p=P)

    col_ch = _chunks(na, 512)        # token column chunks (moving <= 512)
    n_qc = len(col_ch)

    with tile.TileContext(nc) as tc:
        with tc.tile_pool(name="persist", bufs=1) as persist:
            cmneg = persist.tile([P, njt], f32)
            xs = persist.tile([P, DC, na], bf16)
            wq_sb = persist.tile([P, DC, HD], bf16)
            wk_sb = persist.tile([P, DC, HD], bf16)
            wv_sb = persist.tile([P, DC, HD], bf16)
            wo_sb = persist.tile([P, HDT, D], bf16)
            for dc in range(DC):
                nc.sync.dma_start(out=xs[:, dc, :], in_=xs_r[:, dc, :])
            nc.sync.dma_start(out=wq_sb, in_=wq_r)
            nc.sync.dma_start(out=wk_sb, in_=wk_r)
            nc.sync.dma_start(out=cmneg, in_=cm_d.ap())
            nc.sync.dma_start(out=wv_sb, in_=wv_r)
            nc.sync.dma_start(out=wo_sb, in_=wo_r)

            qT = persist.tile([P, HDT, na], bf16)
            kT = persist.tile([P, HDT, na], bf16)
            attnT = persist.tile([P, HDT, na], bf16)
            vaug = persist.tile([P, njt_v, HPC, DHW], bf16)
            pts0 = persist.tile([P, njt_v, na], bf16)
            pts1 = persist.tile([P, njt_v, na], bf16)

            # vaug ones column 64 (softmax-sum row), zero pad cols
            nc.vector.memset(vaug[:, :, :, DH:DHW], 0.0)
            nc.vector.memset(vaug[:, 0:njt, :, DH : DH + 1], 1.0)

            attn_scope = nc.named_scope("main"); attn_scope.__enter__()
            psxp_cm = tc.tile_pool(name="psx", bufs=2, space="PSUM")
            psxp = psxp_cm.__enter__()
            nrm_cm = tc.tile_pool(name="nrm", bufs=3)
            nrm = nrm_cm.__enter__()
            psSp_cm = tc.tile_pool(name="psS", bufs=pss_bufs, space="PSUM")
            psSp = psSp_cm.__enter__()

            def proj_full(w_sb, dst, t):
                """Full-width projection of head-dim chunk t via the big pool."""
                ps = psSp.tile([P, psw], f32, tag="psS", name=f"proj{t}")
                for off, sz in col_ch:
                    for dc in range(DC):
                        nc.tensor.matmul(
                            ps[:, off : off + sz],
                            w_sb[:, dc, t * P : (t + 1) * P],
                            xs[:, dc, off : off + sz],
                            start=(dc == 0),
                            stop=(dc == DC - 1),
                        )
                nc.vector.tensor_copy(dst[:, t, :], ps[:, 0:na])

            # -- PE filler work, emitted piecewise between score tiles --
            filler = []

            def f_vproj(jt):
                def emit():
                    psv = psxp.tile([P, 512], f32, tag="psx", name=f"psv{jt}")
                    for dc in range(DC):
                        nc.tensor.matmul(
                            psv[:, 0:HD],
                            xs[:, dc, jt * P : (jt + 1) * P],
                            wv_sb[:, dc, :],
                            start=(dc == 0),
                            stop=(dc == DC - 1),
                        )
                    nc.vector.tensor_copy(
                        vaug[:, jt, :, 0:DH],
                        psv[:, 0:HD].rearrange("p (h d) -> p h d", h=HPC),
                    )
                return emit

            def f_projchunk(w_sb, dst, t, off, sz):
                def emit():
                    ps = psxp.tile([P, 512], f32, tag="psx",
                                   name=f"pc{t}_{off}")
                    for dc in range(DC):
                        nc.tensor.matmul(
                            ps[:, 0:sz],
                            w_sb[:, dc, t * P : (t + 1) * P],
                            xs[:, dc, off : off + sz],
                            start=(dc == 0),
                            stop=(dc == DC - 1),
                        )
                    nc.vector.tensor_copy(dst[:, t, off : off + sz], ps[:, 0:sz])
                return emit

            def f_vatp(h, i, jt):
                """One P@V accumulation step; allocates psO at jt==0 and
                drains (normalize into attnT) after jt==njt-1."""
                hdt, pb = h // 2, DH * (h % 2)
                pts = pts0 if h % 2 == 0 else pts1
                off, sz = col_ch[i]

                def emit():
                    if jt == 0:
                        psO_state[(h, i)] = psxp.tile(
                            [P, 512], f32, tag="psx", name=f"psO{h}_{i}"
                        )
                    psO = psO_state[(h, i)]
                    nc.tensor.matmul(
                        psO[0:DHW, 0:sz],
                        vaug[:, jt, h, :],
                        pts[:, jt, off : off + sz],
                        start=(jt == 0),
                        stop=(jt == njt - 1),
                    )
                    if jt == njt - 1:
                        # stage s on a fresh partition-0 tile: the custom-DVE
                        # approx reciprocal reads the wrong partition when its
                        # input AP has a partition offset
                        s_c = nrm.tile([1, 512], f32, tag="s_c")
                        nc.vector.tensor_copy(s_c[:, 0:sz], psO[DH : DH + 1, 0:sz])
                        r_row = nrm.tile([1, 512], f32, tag="r_row")
                        nc.vector.reciprocal_approx_fast(
                            out=r_row[:, 0:sz], in_=s_c[:, 0:sz]
                        )
                        rb = nrm.tile([DH, 512], f32, tag="rb")
                        nc.gpsimd.partition_broadcast(
                            rb[:, 0:sz], r_row[:, 0:sz], channels=DH
                        )
                        nc.vector.tensor_mul(
                            attnT[pb : pb + DH, hdt, off : off + sz],
                            psO[0:DH, 0:sz],
                            rb[:, 0:sz],
                        )
                        del psO_state[(h, i)]
                return emit

            psO_state = {}

            # head-dim chunk 0 of Q and K up front; the rest is filler
            proj_full(wq_sb, qT, 0)
            proj_full(wk_sb, kT, 0)
            for jt in range(njt):
                filler.append(f_vproj(jt))
            for t in (1, 2):
                for w_sb, dst in ((wq_sb, qT), (wk_sb, kT)):
                    for off, sz in col_ch:
                        filler.append(f_projchunk(w_sb, dst, t, off, sz))

            # ---------------- attention ----------------
            for h in range(HPC):
                hdt, pb = h // 2, DH * (h % 2)
                pts = pts0 if h % 2 == 0 else pts1
                if h > 0:   # P@V of the previous head becomes filler
                    for i in range(n_qc):
                        for jt in range(njt):
                            filler.append(f_vatp(h - 1, i, jt))
                for jt in range(njt):
                    psS = psSp.tile([P, psw], f32, tag="psS")
                    for off, sz in col_ch:
                        nc.tensor.matmul(
                            psS[:, off : off + sz],
                            kT[pb : pb + DH, hdt, jt * P : (jt + 1) * P],
                            qT[pb : pb + DH, hdt, off : off + sz],
                            start=True,
                            stop=True,
                        )
                    # drip filler to keep PE busy during the ACT-bound loop
                    budget = 3 if h > 0 else 2
                    for _ in range(min(budget, len(filler))):
                        filler.pop(0)()
                    nc.scalar.activation(
                        pts[:, jt, 0:na],
                        psS[:, 0:na],
                        mybir.ActivationFunctionType.Exp,
                        bias=cmneg[:, jt : jt + 1],
                        scale=SCALE,
                    )
            for f in filler:   # leftover filler (rare)
                f()

            psSp_cm.__exit__(None, None, None)

            # ------------- tail: last head's P@V + output projection -------
            with tc.tile_pool(name="psF", bufs=2, space="PSUM") as psFp, \
                 tc.tile_pool(name="fin", bufs=2) as fin:

                def oproj(it):
                    psF = psFp.tile([P, 1024], f32, tag="psF")
                    for off, sz in ((0, 512), (512, 256)):
                        for c in range(HDT):
                            nc.tensor.matmul(
                                psF[:, off : off + sz],
                                attnT[:, c, it * P : (it + 1) * P],
                                wo_sb[:, c, off : off + sz],
                                start=(c == 0),
                                stop=(c == HDT - 1),
                            )
                    out_sb = fin.tile([P, D], f32, tag="out_sb")
                    nc.scalar.copy(out_sb, psF[:, 0:D])
                    nc.sync.dma_start(
                        out=out_d.ap()[it * P : (it + 1) * P, :], in_=out_sb
                    )

                h = HPC - 1
                done_tiles = 0
                for i in range(n_qc):
                    for jt in range(njt):
                        f_vatp(h, i, jt)()
                    if i > 0:
                        # out-proj tiles fully insidenow-drained chunks
                        lim = col_ch[i - 1][0] + col_ch[i - 1][1]
                        while (done_tiles + 1) * P <= lim:
                            oproj(done_tiles)
                            done_tiles += 1
                while done_tiles < njt:
                    oproj(done_tiles)
                    done_tiles += 1

            nrm_cm.__exit__(None, None, None)
            psxp_cm.__exit__(None, None, None)
            attn_scope.__exit__(None, None, None)

    nc.compile()
    _BUILD_CACHE[njt] = nc
    return nc


def _marshal(x, x_mask, Wq, Wk, Wv, Wo):
    """Build per-core input maps. Returns (in_maps, njt, orders, counts)."""
    x = np.asarray(x, dtype=np.float32)
    x_mask = np.asarray(x_mask).astype(bool)
    Wq = np.asarray(Wq, dtype=np.float32)
    Wk = np.asarray(Wk, dtype=np.float32)
    Wv = np.asarray(Wv, dtype=np.float32)
    Wo = np.asarray(Wo, dtype=np.float32)

    orders = [np.argsort(~x_mask[b], kind="stable") for b in range(B)]
    counts = [int(x_mask[b].sum()) for b in range(B)]
    njt = max(1, -(-max(counts) // P))
    na = njt * P

    xsTs, cms = [], []
    for b in range(B):
        xs_sorted = x[b][orders[b][:na]]                 # [na, 768]
        xsTs.append(np.ascontiguousarray(xs_sorted.T.astype(BF16)))
        key_act = np.arange(na) < counts[b]
        cm = np.where(key_act, EXP_SHIFT, MASK_NEG).astype(np.float32)
        cms.append(np.ascontiguousarray(cm.reshape(njt, P).T))

    whs = []
    for hg in range(2):
        cols = slice(hg * HD, (hg + 1) * HD)
        whs.append({
            "wq": np.ascontiguousarray(Wq[:, cols].astype(BF16)),
            "wk": np.ascontiguousarray(Wk[:, cols].astype(BF16)),
            "wv": np.ascontiguousarray(Wv[:, cols].astype(BF16)),
            "woT": np.ascontiguousarray(Wo[cols, :].astype(BF16)),
        })

    in_maps = []
    for c in range(8):
        b, hg = c // 2, c % 2
        in_maps.append({
            "xsT": xsTs[b], "cmneg": cms[b], **whs[hg],
        })
    return in_maps, njt, orders, counts


def run(x, x_mask, Wq, Wk, Wv, Wo, trace=False, tmpdir=None):
    """Run on 8 cores; returns (full_output, BassKernelResults)."""
    x = np.asarray(x, dtype=np.float32)
    Wv_f = np.asarray(Wv, dtype=np.float32)
    Wo_f = np.asarray(Wo, dtype=np.float32)
    in_maps, njt, orders, counts = _marshal(x, x_mask, Wq, Wk, Wv, Wo)
    nc = build(njt)
    res = run_bass_kernel_spmd(
        nc, in_maps, core_ids=list(range(8)), trace=trace, tmpdir=tmpdir
    )
    out = np.empty((B, N, D), dtype=np.float32)
    for b in range(B):
        s = (res.results[2 * b]["out"].astype(np.float32)
             + res.results[2 * b + 1]["out"].astype(np.float32))
        nr = counts[b]
        out[b, orders[b][:nr]] = s[:nr]
        if nr < N:
            # masked queries: uniform softmax over ALL keys
            mu = x[b].astype(np.float64).mean(axis=0)
            urow = (mu @ Wv_f.astype(np.float64)) @ Wo_f.astype(np.float64)
            out[b, orders[b][nr:]] = urow.astype(np.float32)
    return out, res


def kernel(**inputs) -> np.ndarray:
    out, _ = run(
        inputs["x"], inputs["x_mask"],
        inputs["Wq"], inputs["Wk"], inputs["Wv"], inputs["Wo"],
        trace=False,
    )
    return out
